# revision 3
# baseline (speedup 1.0000x reference)
"""CfC cell (dense MLP) Trainium2 Bass kernel.

Reference math (fp32):
    x  = concat([input, hx], axis=1)                  # [B, 768]
    h  = 1.7159 * tanh(0.666 * (x @ Wb.T + bb))       # [B, 1024]
    ff1 = tanh(h @ W1.T + b1)                         # [B, 512]
    ff2 = tanh(h @ W2.T + b2)
    t_a = h @ Wa.T + ba
    t_b = h @ Wt.T + bt
    t   = sigmoid(t_a * ts + t_b)
    out = ff1 * (1 - t) + t * ff2

Strategy: data-parallel over batch across 8 NeuronCores (2048 rows each).
Host-side prep gives the device friendly layouts:
  - xT        [768, 2048]   (x transposed -> contraction dim on partitions)
  - WbT       [768, 1024]   (Wb.T; stationary lhsT tiles for layer 1)
  - WH        [4, 1024, 512] (1.7159 * Wk.T; moving rhs for layer 2)
  - BBP       [128, 8]      (0.666*bb, per unit-tile columns; ACT bias)
  - BH        [4, 128, 512] (head biases broadcast across partitions)
  - TSP       [128, 16]     (ts, column mi = batch subtile mi)
Layer 1 produces hT [units, batch] tiles directly (no on-chip transposes);
layer 2 uses hT slices as the stationary operand producing [batch, hid]
output tiles, so ts becomes a per-partition scalar and the result DMAs out
with no transpose. All matmuls run as float32r (full fp32 storage, 1
cycle/row at N=512 on the PE).
"""

import os
import sys

import numpy as np

if "/opt/trn_rl_repo" not in sys.path:
    sys.path.insert(0, "/opt/trn_rl_repo")

B, IN, HID, UNITS = 16384, 256, 512, 1024
CAT = IN + HID  # 768
N_CORES = 8
BS = B // N_CORES  # 2048 per core
P = 128
NK1 = CAT // P    # 6 contraction tiles, layer 1
NU = UNITS // P   # 8 unit tiles
NH = 4            # heads

_cache = {}


def build_nc(bs=BS, chunk=512):
    """Build the single-core Bass program (same program runs SPMD on 8 cores)."""
    from concourse import bacc, tile, mybir

    AF = mybir.ActivationFunctionType
    ALU = mybir.AluOpType
    F32 = mybir.dt.float32
    F32R = mybir.dt.float32r

    nchunk = bs // chunk
    nm = chunk // P  # batch subtiles per chunk

    nc = bacc.Bacc("TRN2", target_bir_lowering=False, debug=False,
                   num_devices=N_CORES)

    xt_d = nc.dram_tensor("xt", [CAT, bs], F32R, kind="ExternalInput").ap()
    wbt_d = nc.dram_tensor("wbt", [CAT, UNITS], F32R, kind="ExternalInput").ap()
    wh_d = nc.dram_tensor("wh", [NH, UNITS, HID], F32R, kind="ExternalInput").ap()
    bbp_d = nc.dram_tensor("bbp", [P, NU], F32, kind="ExternalInput").ap()
    bh_d = nc.dram_tensor("bh", [NH, P, HID], F32, kind="ExternalInput").ap()
    tsp_d = nc.dram_tensor("tsp", [P, bs // P], F32, kind="ExternalInput").ap()
    out_d = nc.dram_tensor("out", [bs, HID], F32, kind="ExternalOutput").ap()

    with tile.TileContext(nc) as tc:
        with (
            tc.tile_pool(name="const", bufs=1) as const,
            tc.tile_pool(name="xp", bufs=2) as xp,
            tc.tile_pool(name="hp", bufs=2) as hp,
            tc.tile_pool(name="tp", bufs=2) as tp,
            tc.tile_pool(name="op", bufs=3) as op,
            tc.tile_pool(name="ps1", bufs=2, space="PSUM") as ps1p,
            tc.tile_pool(name="ps2", bufs=6, space="PSUM") as ps2p,
        ):
            # --- resident constants -------------------------------------
            wb_sb = []
            for c in range(NK1):
                t = const.tile([P, UNITS], F32R, tag=f"wb{c}")
                nc.sync.dma_start(t[:], wbt_d[c * P:(c + 1) * P, :])
                wb_sb.append(t)
            wh_sb = []
            for k in range(NH):
                row = []
                for u in range(NU):
                    t = const.tile([P, HID], F32R, tag=f"wh{k}_{u}")
                    nc.sync.dma_start(t[:], wh_d[k, u * P:(u + 1) * P, :])
                    row.append(t)
                wh_sb.append(row)
            bb_sb = const.tile([P, NU], F32, tag="bb")
            nc.sync.dma_start(bb_sb[:], bbp_d[:])
            bh_sb = []
            for k in range(NH):
                t = const.tile([P, HID], F32, tag=f"bh{k}")
                nc.sync.dma_start(t[:], bh_d[k])
                bh_sb.append(t)
            ts_sb = const.tile([P, bs // P], F32, tag="ts")
            nc.sync.dma_start(ts_sb[:], tsp_d[:])

            # --- main loop over batch chunks ----------------------------
            for bc in range(nchunk):
                # layer-1 inputs for this chunk
                xts = []
                for c in range(NK1):
                    t = xp.tile([P, chunk], F32R, tag=f"x{c}")
                    nc.sync.dma_start(
                        t[:], xt_d[c * P:(c + 1) * P, bc * chunk:(bc + 1) * chunk])
                    xts.append(t)

                # layer 1: hT[u] = tanh(0.666*(WbT.T @ xT) + 0.666*bb)
                hts = []
                for u in range(NU):
                    ps = ps1p.tile([P, chunk], F32)
                    for c in range(NK1):
                        nc.tensor.matmul(
                            ps[:],
                            wb_sb[c][:, u * P:(u + 1) * P],
                            xts[c][:],
                            start=(c == 0), stop=(c == NK1 - 1))
                    ht = hp.tile([P, chunk], F32R, tag=f"h{u}")
                    nc.scalar.activation(ht[:], ps[:], AF.Tanh,
                                         bias=bb_sb[:, u:u + 1], scale=0.666)
                    hts.append(ht)

                # layer 2 + elementwise, per 128-row batch subtile
                for m in range(nm):
                    mi = bc * nm + m
                    pss = []
                    for k in range(NH):
                        ps = ps2p.tile([P, HID], F32)
                        for u in range(NU):
                            nc.tensor.matmul(
                                ps[:],
                                hts[u][:, m * P:(m + 1) * P],
                                wh_sb[k][u][:],
                                start=(u == 0), stop=(u == NU - 1))
                        pss.append(ps)
                    p1, p2, pa, pb = pss

                    u1 = tp.tile([P, HID], F32, tag="u1")
                    nc.vector.tensor_add(u1[:], p1[:], bh_sb[0][:])
                    f1 = tp.tile([P, HID], F32, tag="f1")
                    nc.scalar.activation(f1[:], u1[:], AF.Tanh)

                    u2 = tp.tile([P, HID], F32, tag="u2")
                    nc.vector.tensor_add(u2[:], p2[:], bh_sb[1][:])
                    f2 = tp.tile([P, HID], F32, tag="f2")
                    nc.scalar.activation(f2[:], u2[:], AF.Tanh)

                    ua = tp.tile([P, HID], F32, tag="ua")
                    nc.vector.tensor_add(ua[:], pa[:], bh_sb[2][:])
                    ub = tp.tile([P, HID], F32, tag="ub")
                    nc.vector.tensor_add(ub[:], pb[:], bh_sb[3][:])

                    w = tp.tile([P, HID], F32, tag="w")
                    nc.vector.scalar_tensor_tensor(
                        w[:], ua[:], ts_sb[:, mi:mi + 1], ub[:],
                        op0=ALU.mult, op1=ALU.add)
                    tt = tp.tile([P, HID], F32, tag="tt")
                    nc.scalar.activation(tt[:], w[:], AF.Sigmoid)

                    # o = f1 + tt*(f2 - f1)
                    o = op.tile([P, HID], F32, tag="o")
                    nc.vector.tensor_sub(o[:], f2[:], f1[:])
                    nc.vector.tensor_mul(o[:], o[:], tt[:])
                    nc.vector.tensor_add(o[:], o[:], f1[:])

                    nc.sync.dma_start(out_d[mi * P:(mi + 1) * P, :], o[:])

    nc.compile()
    return nc


def _prep_inputs(input, hx, ts, Wb, bb, W1, b1, W2, b2, Wa, ba, Wt, bt, bs=BS,
                 n_cores=N_CORES):
    f = np.float32
    x = np.concatenate([np.asarray(input, f), np.asarray(hx, f)], axis=1)
    WbT = np.ascontiguousarray(np.asarray(Wb, f).T)          # [768, 1024]
    WH = np.stack([np.ascontiguousarray((1.7159 * np.asarray(W, f)).T)
                   for W in (W1, W2, Wa, Wt)])               # [4, 1024, 512]
    BBP = np.ascontiguousarray(
        (0.666 * np.asarray(bb, f)).reshape(NU, P).T)        # [128, 8]
    BH = np.stack([np.ascontiguousarray(np.broadcast_to(np.asarray(b, f), (P, HID)))
                   for b in (b1, b2, ba, bt)])               # [4, 128, 512]
    ts = np.asarray(ts, f).reshape(-1)

    in_maps = []
    for c in range(n_cores):
        lo, hi = c * bs, (c + 1) * bs
        in_maps.append({
            "xt": np.ascontiguousarray(x[lo:hi].T),          # [768, bs]
            "wbt": WbT,
            "wh": WH,
            "bbp": BBP,
            "bh": BH,
            "tsp": np.ascontiguousarray(ts[lo:hi].reshape(bs // P, P).T),
        })
    return in_maps


def kernel(input, hx, ts, Wb, bb, W1, b1, W2, b2, Wa, ba, Wt, bt):
    from concourse.bass_utils import run_bass_kernel_spmd

    if "nc" not in _cache:
        _cache["nc"] = build_nc()
    nc = _cache["nc"]

    in_maps = _prep_inputs(input, hx, ts, Wb, bb, W1, b1, W2, b2, Wa, ba, Wt, bt)
    trace = bool(int(os.environ.get("KERNEL_PROFILE", "0")))
    res = run_bass_kernel_spmd(nc, in_maps, list(range(N_CORES)), trace=trace)
    _cache["last_exec_time_ns"] = res.exec_time_ns
    _cache["last_results"] = res

    out = np.concatenate([res.results[c]["out"] for c in range(N_CORES)], axis=0)
    return out.astype(np.float32)


# revision 4
# speedup vs baseline: 1.2135x; 1.2135x over previous
"""CfC cell (dense MLP) Trainium2 Bass kernel.

Reference math (fp32):
    x  = concat([input, hx], axis=1)                  # [B, 768]
    h  = 1.7159 * tanh(0.666 * (x @ Wb.T + bb))       # [B, 1024]
    ff1 = tanh(h @ W1.T + b1)                         # [B, 512]
    ff2 = tanh(h @ W2.T + b2)
    t_a = h @ Wa.T + ba
    t_b = h @ Wt.T + bt
    t   = sigmoid(t_a * ts + t_b)
    out = ff1 * (1 - t) + t * ff2

Strategy: data-parallel over batch across 8 NeuronCores (2048 rows each).
Host-side prep gives the device friendly layouts (fp16 matmul operands,
fp32 accumulation and elementwise):
  - xT        [768, 2048]   (x transposed -> contraction dim on partitions)
  - WbT       [768, 1024]   (Wb.T; stationary lhsT tiles for layer 1)
  - WH        [4, 1024, 512] (1.7159 * Wk.T; moving rhs for layer 2)
  - BBP       [128, 8]      (0.666*bb, per unit-tile columns; ACT bias)
  - BH        [4, 128, 512] (head biases broadcast across partitions)
  - TSP       [128, 16]     (ts, column mi = batch subtile mi)
Layer 1 produces hT [units, batch] tiles directly (no on-chip transposes);
layer 2 uses hT slices as the stationary operand producing [batch, hid]
output tiles, so ts becomes a per-partition scalar and the result DMAs out
with no transpose. Layer-1 runs one chunk ahead of layer-2 so the PE never
waits on the head-weight DMAs during startup.
"""

import os
import sys

import numpy as np

if "/opt/trn_rl_repo" not in sys.path:
    sys.path.insert(0, "/opt/trn_rl_repo")

B, IN, HID, UNITS = 16384, 256, 512, 1024
CAT = IN + HID  # 768
N_CORES = 8
BS = B // N_CORES  # 2048 per core
P = 128
NK1 = CAT // P    # 6 contraction tiles, layer 1
NU = UNITS // P   # 8 unit tiles
NH = 4            # heads

_cache = {}


def build_nc(bs=BS, chunk=512):
    """Build the single-core Bass program (same program runs SPMD on 8 cores)."""
    from concourse import bacc, tile, mybir

    AF = mybir.ActivationFunctionType
    ALU = mybir.AluOpType
    F32 = mybir.dt.float32
    F16 = mybir.dt.float16

    nchunk = bs // chunk
    nm = chunk // P  # batch subtiles per chunk

    nc = bacc.Bacc("TRN2", target_bir_lowering=False, debug=False,
                   num_devices=N_CORES)

    xt_d = nc.dram_tensor("xt", [CAT, bs], F16, kind="ExternalInput").ap()
    wbt_d = nc.dram_tensor("wbt", [CAT, UNITS], F16, kind="ExternalInput").ap()
    wh_d = nc.dram_tensor("wh", [NH, UNITS, HID], F16, kind="ExternalInput").ap()
    bbp_d = nc.dram_tensor("bbp", [P, NU], F32, kind="ExternalInput").ap()
    bh_d = nc.dram_tensor("bh", [NH, P, HID], F32, kind="ExternalInput").ap()
    tsp_d = nc.dram_tensor("tsp", [P, bs // P], F32, kind="ExternalInput").ap()
    out_d = nc.dram_tensor("out", [bs, HID], F32, kind="ExternalOutput").ap()

    with tile.TileContext(nc) as tc:
        with (
            tc.tile_pool(name="const", bufs=1) as const,
            tc.tile_pool(name="xp", bufs=2) as xp,
            tc.tile_pool(name="hp", bufs=2) as hp,
            tc.tile_pool(name="tp", bufs=2) as tp,
            tc.tile_pool(name="op", bufs=3) as op,
            tc.tile_pool(name="ps1", bufs=2, space="PSUM") as ps1p,
            tc.tile_pool(name="ps2", bufs=6, space="PSUM") as ps2p,
        ):
            # --- layer-1 constants first (unblock the first matmuls) -----
            wb_sb = []
            for c in range(NK1):
                t = const.tile([P, UNITS], F16, tag=f"wb{c}")
                nc.sync.dma_start(t[:], wbt_d[c * P:(c + 1) * P, :])
                wb_sb.append(t)
            bb_sb = const.tile([P, NU], F32, tag="bb")
            nc.sync.dma_start(bb_sb[:], bbp_d[:])

            def load_x(bc):
                xts = []
                for c in range(NK1):
                    t = xp.tile([P, chunk], F16, tag=f"x{c}")
                    nc.sync.dma_start(
                        t[:], xt_d[c * P:(c + 1) * P, bc * chunk:(bc + 1) * chunk])
                    xts.append(t)
                return xts

            xts0 = load_x(0)

            # --- layer-2 constants (stream in behind layer-1 work) -------
            wh_sb = []
            for k in range(NH):
                row = []
                for u in range(NU):
                    t = const.tile([P, HID], F16, tag=f"wh{k}_{u}")
                    nc.sync.dma_start(t[:], wh_d[k, u * P:(u + 1) * P, :])
                    row.append(t)
                wh_sb.append(row)
            bh_sb = []
            for k in range(NH):
                t = const.tile([P, HID], F32, tag=f"bh{k}")
                nc.sync.dma_start(t[:], bh_d[k])
                bh_sb.append(t)
            ts_sb = const.tile([P, bs // P], F32, tag="ts")
            nc.sync.dma_start(ts_sb[:], tsp_d[:])

            def layer1(xts):
                """hT[u] = tanh(0.666*(WbT.T @ xT) + 0.666*bb), fp16 out."""
                hts = []
                for u in range(NU):
                    ps = ps1p.tile([P, chunk], F32)
                    for c in range(NK1):
                        nc.tensor.matmul(
                            ps[:],
                            wb_sb[c][:, u * P:(u + 1) * P],
                            xts[c][:],
                            start=(c == 0), stop=(c == NK1 - 1))
                    ht = hp.tile([P, chunk], F16, tag=f"h{u}")
                    nc.scalar.activation(ht[:], ps[:], AF.Tanh,
                                         bias=bb_sb[:, u:u + 1], scale=0.666)
                    hts.append(ht)
                return hts

            def layer2(hts, bc):
                for m in range(nm):
                    mi = bc * nm + m
                    pss = []
                    for k in range(NH):
                        ps = ps2p.tile([P, HID], F32)
                        for u in range(NU):
                            nc.tensor.matmul(
                                ps[:],
                                hts[u][:, m * P:(m + 1) * P],
                                wh_sb[k][u][:],
                                start=(u == 0), stop=(u == NU - 1))
                        pss.append(ps)
                    p1, p2, pa, pb = pss

                    u1 = tp.tile([P, HID], F32, tag="u1")
                    nc.vector.tensor_add(u1[:], p1[:], bh_sb[0][:])
                    f1 = tp.tile([P, HID], F32, tag="f1")
                    nc.scalar.activation(f1[:], u1[:], AF.Tanh)

                    u2 = tp.tile([P, HID], F32, tag="u2")
                    nc.vector.tensor_add(u2[:], p2[:], bh_sb[1][:])
                    f2 = tp.tile([P, HID], F32, tag="f2")
                    nc.scalar.activation(f2[:], u2[:], AF.Tanh)

                    ua = tp.tile([P, HID], F32, tag="ua")
                    nc.vector.tensor_add(ua[:], pa[:], bh_sb[2][:])
                    ub = tp.tile([P, HID], F32, tag="ub")
                    nc.vector.tensor_add(ub[:], pb[:], bh_sb[3][:])

                    w = tp.tile([P, HID], F32, tag="w")
                    nc.vector.scalar_tensor_tensor(
                        w[:], ua[:], ts_sb[:, mi:mi + 1], ub[:],
                        op0=ALU.mult, op1=ALU.add)
                    tt = tp.tile([P, HID], F32, tag="tt")
                    nc.scalar.activation(tt[:], w[:], AF.Sigmoid)

                    # o = f1 + tt*(f2 - f1)
                    o = op.tile([P, HID], F32, tag="o")
                    nc.vector.tensor_sub(o[:], f2[:], f1[:])
                    nc.vector.tensor_mul(o[:], o[:], tt[:])
                    nc.vector.tensor_add(o[:], o[:], f1[:])

                    nc.sync.dma_start(out_d[mi * P:(mi + 1) * P, :], o[:])

            # --- software pipeline: layer-1 one chunk ahead --------------
            hts_cur = layer1(xts0)
            for bc in range(nchunk):
                if bc + 1 < nchunk:
                    hts_next = layer1(load_x(bc + 1))
                else:
                    hts_next = None
                layer2(hts_cur, bc)
                hts_cur = hts_next

    nc.compile()
    return nc


def _prep_inputs(input, hx, ts, Wb, bb, W1, b1, W2, b2, Wa, ba, Wt, bt, bs=BS,
                 n_cores=N_CORES):
    f = np.float32
    h = np.float16
    x = np.concatenate([np.asarray(input, f), np.asarray(hx, f)], axis=1)
    WbT = np.ascontiguousarray(np.asarray(Wb, f).T.astype(h))   # [768, 1024]
    WH = np.stack([np.ascontiguousarray((1.7159 * np.asarray(W, f)).T.astype(h))
                   for W in (W1, W2, Wa, Wt)])                  # [4, 1024, 512]
    BBP = np.ascontiguousarray(
        (0.666 * np.asarray(bb, f)).reshape(NU, P).T)           # [128, 8]
    BH = np.stack([np.ascontiguousarray(np.broadcast_to(np.asarray(b, f), (P, HID)))
                   for b in (b1, b2, ba, bt)])                  # [4, 128, 512]
    ts = np.asarray(ts, f).reshape(-1)
    xh = x.astype(h)

    in_maps = []
    for c in range(n_cores):
        lo, hi = c * bs, (c + 1) * bs
        in_maps.append({
            "xt": np.ascontiguousarray(xh[lo:hi].T),            # [768, bs] fp16
            "wbt": WbT,
            "wh": WH,
            "bbp": BBP,
            "bh": BH,
            "tsp": np.ascontiguousarray(ts[lo:hi].reshape(bs // P, P).T),
        })
    return in_maps


def kernel(input, hx, ts, Wb, bb, W1, b1, W2, b2, Wa, ba, Wt, bt):
    from concourse.bass_utils import run_bass_kernel_spmd

    if "nc" not in _cache:
        _cache["nc"] = build_nc()
    nc = _cache["nc"]

    in_maps = _prep_inputs(input, hx, ts, Wb, bb, W1, b1, W2, b2, Wa, ba, Wt, bt)
    trace = bool(int(os.environ.get("KERNEL_PROFILE", "0")))
    res = run_bass_kernel_spmd(nc, in_maps, list(range(N_CORES)), trace=trace)
    _cache["last_exec_time_ns"] = res.exec_time_ns
    _cache["last_results"] = res

    out = np.concatenate([res.results[c]["out"] for c in range(N_CORES)], axis=0)
    return out.astype(np.float32)
